# revision 22
# baseline (speedup 1.0000x reference)
"""BiaffineSpan TRN2 kernel — v4: bandwidth-aware lead-in + tail polish.

Full-input contract: kernel(**inputs) -> [B, L, L, C] float32.

Sharding: the C=256 bilinear channel dim is split across 8 NeuronCores
(32 channels each).

Algebra: with Hs = h1s@sw2.T + sb2, h1s = relu(x@sw1.T + sb1) (same for
the end MLP), the bilinear einsum Hs @ W1c @ He.T collapses to

    h1s @ (sw2.T @ W1c @ ew2) @ h1e.T  (+ bias terms)

so each device only runs the two ReLU layer-1 GEMMs; the layer-2
weights are folded into precomputed What[c] = sw2.T @ W1c @ ew2 and the
bias cross-terms fold exactly into the Ls / Le / w0 linear terms.

Everything on-device runs in bf16 (fp32 PSUM); the output is written
bf16 (halves the 67MB/core output DMA) and the host upcasts.

v4 lead-in model (trace-driven): Tile gates a consumer on ALL writes to
a tile, and the ~358GB/s HBM link is the real constraint early on.  So:
- ew1 arrives as six separate per-ot-block TILES (ot-major host
  layout): the first linear1 chain only gates on xA + ot-block 0
  (~1.4MB) instead of the whole 2.4MB input set;
- all big non-critical streams (xB, sw1T, What prefetches) share the
  gpsimd queue BEHIND xA, so per-queue FIFO keeps them off the
  critical bandwidth window;
- warmup is ~40 N=128 matmuls on a zero tile - enough busy time that
  the HAM clock-gate is released before the real matmuls start.
Tail: the final (c,b) stage2 runs half-width (N=256) chains with its
output DMAs split across two queues, halving the post-matmul drain.
"""

import numpy as np
import ml_dtypes
from contextlib import ExitStack

import concourse.bass as bass
import concourse.bacc as bacc
import concourse.mybir as mybir
import concourse.tile as tile
from concourse.bass_utils import run_bass_kernel_spmd

B, L, D, C = 2, 512, 768, 256
NCORES = 8
CLOC = C // NCORES          # 32 channels per core
T = B * L                   # 1024 tokens
P = 128
DT = D // P                 # 6 feature tiles
LT = L // P                 # 4 token tiles per batch el
TT = T // P                 # 8 token tiles total
NCH = T // 512              # 2 moving chunks of 512 tokens
NWARM = 15

F32 = mybir.dt.float32
BF = mybir.dt.bfloat16
NBF = ml_dtypes.bfloat16
MODE = "v4-bf16"


def build_program(cloc=CLOC):
    nc = bacc.Bacc("TRN2", target_bir_lowering=False, debug=False)

    # All inputs pre-arranged host-side to [partition, free...] so every
    # DMA is contiguous >=4KB per partition row.
    xTa_h = nc.declare_dram_parameter("xTa", [P, DT * 512], BF, isOutput=False)
    xTb_h = nc.declare_dram_parameter("xTb", [P, DT * 512], BF, isOutput=False)
    sw1T_h = nc.declare_dram_parameter("sw1T", [P, DT * D], BF, isOutput=False)
    # ew1o: ot-major layout [P, ot, kt, 128]
    ew1o_h = nc.declare_dram_parameter("ew1o", [P, DT * D], BF, isOutput=False)
    sb1_h = nc.declare_dram_parameter("sb1", [P, DT], F32, isOutput=False)
    eb1_h = nc.declare_dram_parameter("eb1", [P, DT], F32, isOutput=False)
    what_h = nc.declare_dram_parameter("what", [cloc, P, DT * D], BF, isOutput=False)
    wsT_h = nc.declare_dram_parameter("wsT", [P, DT * cloc], BF, isOutput=False)
    weT_h = nc.declare_dram_parameter("weT", [P, DT * cloc], BF, isOutput=False)
    w0_h = nc.declare_dram_parameter("w0", [cloc, 1], F32, isOutput=False)
    out_h = nc.declare_dram_parameter("out", [B, cloc, L, L], BF, isOutput=True)
    # DRAM bounce for Le rows, so they can be partition-broadcast back in
    leD_h = nc.dram_tensor("leD", [cloc, T], BF)

    Relu = mybir.ActivationFunctionType.Relu
    Ident = mybir.ActivationFunctionType.Identity

    def mm(ps, lhsT, rhs, start, stop):
        nc.tensor.matmul(ps, lhsT, rhs, start=start, stop=stop)

    with tile.TileContext(nc) as tc, ExitStack() as ctx:
        # persistent pools
        p_h = ctx.enter_context(tc.tile_pool(name="hids", bufs=1))
        p_lin = ctx.enter_context(tc.tile_pool(name="lin", bufs=1))

        h1sT = p_h.tile([P, DT, T], BF, tag="hs")
        h1eT = p_h.tile([P, DT, T], BF, tag="he")
        p_w1 = ctx.enter_context(tc.tile_pool(name="w1c", bufs=2))
        what_ap = what_h[:]

        def w1_prefetch(c):
            t = p_w1.tile([P, DT, D], BF, tag="w1t", name="w1t")
            nc.gpsimd.dma_start(out=t[:], in_=what_ap[c])
            return t

        # -------- Phase A0: PE warmup during the DMA lead-in --------
        # N=256 matmuls on a zero tile: ~5.5us of continuous PE busy so
        # the HAM clock-gate (needs ~50 matmuls / >3.4us of activity)
        # has released by the time the critical inputs land (~12.5us).
        # The memset runs on the Vector queue so the gpsimd queue's
        # first instruction is the xA DMA issue.
        wz = p_h.tile([P, 2 * P], BF, tag="wz")
        with tc.tile_pool(name="ps_w", bufs=1, space="PSUM") as ps_w:
            nc.vector.memset(wz[:], 0)
            psw = ps_w.tile([P, 2 * P], F32, tag="psw")
            for i in range(NWARM):
                mm(psw[:], wz[:, :P], wz[:], start=True, stop=True)

        # ---------------- Phase A: layer-1 of both MLPs ----------------
        with (
            tc.tile_pool(name="ph_a", bufs=1) as p_a,
            tc.tile_pool(name="ps_a", bufs=4, space="PSUM") as ps_a,
            tc.tile_pool(name="ps_b", bufs=2, space="PSUM") as ps_b,
            tc.tile_pool(name="bias", bufs=1) as p_bias,
        ):
            # Critical path: xA (gpsimd) + ew1 ot-block tiles (scalar)
            # stream concurrently; everything else queues behind xA on
            # gpsimd so it cannot steal the early HBM bandwidth.
            xA0 = p_a.tile([P, DT, 256], BF, tag="xa0")
            xA1 = p_a.tile([P, DT, 256], BF, tag="xa1")
            xB = p_a.tile([P, DT, 512], BF, tag="xb")
            # ew1 as three 2-ot-block tiles: the first linear1 chain only
            # gates on xA + block 0 instead of the whole weight matrix.
            ew1 = [p_a.tile([P, 2, DT, P], BF, tag=f"we{g}", name=f"we{g}")
                   for g in range(DT // 2)]
            sw1T = p_a.tile([P, DT, D], BF, tag="sw")
            ew1o_v = ew1o_h[:].rearrange("p (g o t n) -> p g o t n",
                                         g=DT // 2, o=2, t=DT)
            b_sb = {}
            # Everything on ONE queue: per-queue FIFO transfer order is
            # the only reliable HBM-bandwidth priority control.  xA comes
            # as two token-halves (contiguous in the host layout) so the
            # first chains gate on 590KB only.
            xTa_v = xTa_h[:].rearrange("p (c t n) -> p c t n", c=2, n=256)
            nc.gpsimd.dma_start(out=xA0[:], in_=xTa_v[:, 0])
            nc.gpsimd.dma_start(out=xA1[:], in_=xTa_v[:, 1])
            for g in range(DT // 2):
                nc.gpsimd.dma_start(out=ew1[g][:], in_=ew1o_v[:, g])
                if g == 0:
                    for nm, h in (("eb1", eb1_h), ("sb1", sb1_h)):
                        b_sb[nm] = p_bias.tile([P, DT], F32, tag=nm, name=nm)
                        nc.scalar.dma_start(out=b_sb[nm][:], in_=h[:])
            nc.gpsimd.dma_start(out=xB[:], in_=xTb_h[:])
            nc.gpsimd.dma_start(out=sw1T[:], in_=sw1T_h[:])

            def linear1(wt_of, bias_t, outT):
                # chunk 0 runs half-token chains (h-major) so the first
                # chain issues as soon as xA0 lands; chunk 1 full-width
                parts = [(xA0, 0, 256), (xA1, 256, 256), (xB, 512, 512)]
                for xt, off, w in parts:
                    for ot in range(DT):
                        ps = ps_a.tile([P, 512], F32, tag="ps_mlp")
                        for kt in range(DT):
                            mm(ps[:, :w],
                               wt_of(ot, kt),
                               xt[:, kt, :],
                               start=(kt == 0), stop=(kt == DT - 1))
                        nc.scalar.activation(
                            outT[:, ot, off:off + w], ps[:, :w],
                            Relu, bias=bias_t[:, ot:ot + 1])

            # LsP [tok_tile, 128, cloc]  (token on partitions, channel free)
            # LeT [cloc, T] (+ w0)      (channel on partitions, token free)
            lsP = p_lin.tile([P, TT, cloc], F32, tag="lsP")
            leT = p_lin.tile([cloc, T], BF, tag="leT")
            wsT = p_bias.tile([P, DT, cloc], BF, tag="ws")
            weT = p_bias.tile([P, DT, cloc], BF, tag="we")
            w0_sb = p_bias.tile([cloc, 1], F32, tag="w0")

            # End-side first so the Le DRAM bounce round-trip is covered
            # long before the first stage2 consumes it.
            linear1(lambda ot, kt: ew1[ot // 2][:, ot % 2, kt, :],
                    b_sb["eb1"], h1eT)
            nc.scalar.dma_start(out=weT[:], in_=weT_h[:])
            nc.scalar.dma_start(out=w0_sb[:], in_=w0_h[:])
            for chk in range(NCH):
                ps = ps_b.tile([cloc, 512], F32, tag="ps_le")
                for kt in range(DT):
                    mm(ps[:],
                       weT[:, kt, :],
                       h1eT[:, kt, chk * 512:(chk + 1) * 512],
                       start=(kt == 0), stop=(kt == DT - 1))
                nc.scalar.activation(
                    leT[:, chk * 512:(chk + 1) * 512], ps[:], Ident,
                    bias=w0_sb[:, 0:1])
            nc.sync.dma_start(out=leD_h[:], in_=leT[:])

            linear1(lambda ot, kt: sw1T[:, kt, ot * P:(ot + 1) * P],
                    b_sb["sb1"], h1sT)
            nc.scalar.dma_start(out=wsT[:], in_=wsT_h[:])
            w1_pref = [w1_prefetch(0), w1_prefetch(1)]
            for tt_ in range(TT):
                ps = ps_b.tile([P, cloc], F32, tag="ps_ls")
                for kt in range(DT):
                    mm(ps[:],
                       h1sT[:, kt, tt_ * P:(tt_ + 1) * P],
                       wsT[:, kt, :],
                       start=(kt == 0), stop=(kt == DT - 1))
                nc.scalar.activation(lsP[:, tt_, :cloc], ps[:], Ident)

        # ---------------- Phase C: main biaffine loop ----------------
        with (
            tc.tile_pool(name="ttp", bufs=2) as p_tt,
            tc.tile_pool(name="lebp", bufs=3) as p_leb,
            tc.tile_pool(name="outp", bufs=6) as p_out,
            tc.tile_pool(name="ps_s1", bufs=4, space="PSUM") as ps_s1,
            tc.tile_pool(name="ps_s2", bufs=4, space="PSUM") as ps_s2,
        ):
            out_ap = out_h[:]

            def stage1(w1t, b):
                tt_t = p_tt.tile([P, DT, 512], BF, tag="tt", name="tt_t")
                for et in range(DT):
                    ps = ps_s1.tile([P, 512], F32, tag="s1", name="ps1")
                    for dt_ in range(DT):
                        mm(ps[:],
                           w1t[:, dt_, et * P:(et + 1) * P],
                           h1sT[:, dt_, b * 512:(b + 1) * 512],
                           start=(dt_ == 0), stop=(dt_ == DT - 1))
                    nc.vector.tensor_copy(tt_t[:, et, :], ps[:])
                return tt_t

            def stage2(tt_t, c, b, last=False):
                # Le[b,:,c] + w0[c], broadcast across the 128 i-partitions
                leB = p_leb.tile([P, 512], BF, tag="leB", name="leB")
                nc.sync.dma_start(
                    out=leB[:],
                    in_=leD_h[c, b * 512:(b + 1) * 512].partition_broadcast(P))
                # the final iteration runs half-width chains so the
                # post-matmul eviction+DMA tail is half as deep
                nh = 2 if last else 1
                nw = 512 // nh
                for it in range(LT):
                    for h in range(nh):
                        ps2 = ps_s2.tile([P, nw], F32, tag="s2", name="ps2")
                        j0 = b * 512 + h * nw
                        for et in range(DT):
                            mm(ps2[:],
                               tt_t[:, et, it * P:(it + 1) * P],
                               h1eT[:, et, j0:j0 + nw],
                               start=(et == 0), stop=(et == DT - 1))
                        o_t = p_out.tile([P, nw], BF, tag="o", name="o_t")
                        # pass 1 (ACT): psum + Ls -> o_t ; pass 2 (DVE): += Le
                        nc.scalar.activation(
                            o_t[:], ps2[:], Ident,
                            bias=lsP[:, b * LT + it, c:c + 1])
                        nc.vector.tensor_add(
                            o_t[:], o_t[:], leB[:, h * nw:h * nw + nw])
                        q = (nc.sync, nc.scalar)[(it * nh + h) % 2]
                        q.dma_start(
                            out=out_ap[b, c, it * P:(it + 1) * P,
                                       h * nw:h * nw + nw],
                            in_=o_t[:])

            # Software-pipelined: stage1 of iteration k+1 is emitted before
            # stage2 of iteration k so PE never waits on TT evictions.
            pending = None
            for c in range(cloc):
                w1t = w1_pref.pop(0)
                if c + 2 < cloc:
                    w1_pref.append(w1_prefetch(c + 2))
                for b in range(B):
                    tt_t = stage1(w1t, b)
                    if pending is not None:
                        stage2(*pending)
                    pending = (tt_t, c, b)
            stage2(*pending, last=True)
    nc.finalize()
    return nc


def _prep_inputs(inputs, cloc=CLOC, ncores=NCORES):
    """Host-side: absorb layer-2 weights, transpose/cast/shard."""
    f32 = np.float32

    def b16(x):
        return np.ascontiguousarray(np.asarray(x, f32), dtype=NBF)

    def parr(a):
        """[D, N] -> [P, DT*N]: partition-major pre-arranged layout."""
        n = a.shape[1]
        return np.ascontiguousarray(
            a.reshape(DT, P, n).transpose(1, 0, 2).reshape(P, DT * n))

    h = np.asarray(inputs["hidden_states"], f32)
    xT3 = parr(b16(h.reshape(T, D).T)).reshape(P, DT, T)
    # xTa: two contiguous token-halves [xA0 | xA1], each [P, DT*256]
    xTa = np.concatenate(
        [np.ascontiguousarray(xT3[:, :, o:o + 256].reshape(P, DT * 256))
         for o in (0, 256)], axis=1)
    xTb = np.ascontiguousarray(xT3[:, :, 512:].reshape(P, DT * 512))
    sw1T = parr(b16(np.asarray(inputs["sw1"], f32).T))
    # ew1 in ot-major layout: [P, ot, kt, 128]
    ew1_lhsT = b16(np.asarray(inputs["ew1"], f32).T)           # [D_in, D_out]
    ew1o = np.ascontiguousarray(
        ew1_lhsT.reshape(DT, P, DT, P).transpose(1, 2, 0, 3)
        .reshape(P, DT * D))
    sb1 = parr(np.asarray(inputs["sb1"], f32).reshape(D, 1).copy())
    eb1 = parr(np.asarray(inputs["eb1"], f32).reshape(D, 1).copy())
    sw2 = np.asarray(inputs["sw2"], f32)
    ew2 = np.asarray(inputs["ew2"], f32)
    sb2 = np.asarray(inputs["sb2"], f32)
    eb2 = np.asarray(inputs["eb2"], f32)
    W1 = np.asarray(inputs["W1"], f32)
    W2w = np.asarray(inputs["W2_w"], f32)
    Ws, We = W2w[:, :D], W2w[:, D:]

    # Absorb: What[c] = sw2.T @ W1[c] @ ew2, with exact bias folds.
    A = np.matmul(sw2.T[None, :, :], W1)           # [C, D, D]
    What = np.matmul(A, ew2[None, :, :])           # [C, D, D]
    wsT = sw2.T @ Ws.T + (A @ eb2).T               # [D, C]
    weT = ew2.T @ We.T + np.einsum('d,cde->ec', sb2, np.matmul(W1, ew2[None]))
    w0 = (np.asarray(inputs["W2_b"], f32) + np.asarray(inputs["bias"], f32)
          + np.einsum('d,cde,e->c', sb2, W1, eb2)
          + Ws @ sb2 + We @ eb2).reshape(C, 1)

    # What[c]: lhsT layout is [d, e] = What[c] itself; pre-arrange d into
    # [P, DT*D] partition-major per channel.
    What16 = np.ascontiguousarray(
        What.astype(NBF).reshape(C, DT, P, D).transpose(0, 2, 1, 3)
        .reshape(C, P, DT * D))
    wsT16 = parr(b16(wsT))                         # [P, DT*C]
    weT16 = parr(b16(weT))

    wsT16 = wsT16.reshape(P, DT, C)
    weT16 = weT16.reshape(P, DT, C)

    in_maps = []
    for m in range(ncores):
        cs = slice(m * cloc, (m + 1) * cloc)
        in_maps.append({
            "xTa": xTa, "xTb": xTb, "sw1T": sw1T, "ew1o": ew1o,
            "sb1": sb1, "eb1": eb1,
            "what": np.ascontiguousarray(What16[cs]),
            "wsT": np.ascontiguousarray(wsT16[:, :, cs].reshape(P, DT * cloc)),
            "weT": np.ascontiguousarray(weT16[:, :, cs].reshape(P, DT * cloc)),
            "w0": np.ascontiguousarray(w0[cs].astype(f32)),
        })
    return in_maps


def _gather(per_core_outs):
    full = np.concatenate([o.astype(np.float32) for o in per_core_outs],
                          axis=1)                  # [B, C, L, L]
    return np.ascontiguousarray(full.transpose(0, 2, 3, 1))  # [B, L, L, C]


def kernel(**inputs):
    in_maps = _prep_inputs(inputs)
    nc = build_program()
    res = run_bass_kernel_spmd(nc, in_maps, list(range(NCORES)))
    return _gather([np.asarray(r["out"]) for r in res.results])


# revision 28
# speedup vs baseline: 1.0053x; 1.0053x over previous
"""BiaffineSpan TRN2 kernel — v4: bandwidth-aware lead-in + tail polish.

Full-input contract: kernel(**inputs) -> [B, L, L, C] float32.

Sharding: the C=256 bilinear channel dim is split across 8 NeuronCores
(32 channels each).

Algebra: with Hs = h1s@sw2.T + sb2, h1s = relu(x@sw1.T + sb1) (same for
the end MLP), the bilinear einsum Hs @ W1c @ He.T collapses to

    h1s @ (sw2.T @ W1c @ ew2) @ h1e.T  (+ bias terms)

so each device only runs the two ReLU layer-1 GEMMs; the layer-2
weights are folded into precomputed What[c] = sw2.T @ W1c @ ew2 and the
bias cross-terms fold exactly into the Ls / Le / w0 linear terms.

Everything on-device runs in bf16 (fp32 PSUM); the output is written
bf16 (halves the 67MB/core output DMA) and the host upcasts.

v4 lead-in model (trace-driven): Tile gates a consumer on ALL writes to
a tile, and the ~358GB/s HBM link is the real constraint early on.  So:
- ew1 arrives as six separate per-ot-block TILES (ot-major host
  layout): the first linear1 chain only gates on xA + ot-block 0
  (~1.4MB) instead of the whole 2.4MB input set;
- all big non-critical streams (xB, sw1T, What prefetches) share the
  gpsimd queue BEHIND xA, so per-queue FIFO keeps them off the
  critical bandwidth window;
- warmup is ~40 N=128 matmuls on a zero tile - enough busy time that
  the HAM clock-gate is released before the real matmuls start.
Tail: the final (c,b) stage2 runs half-width (N=256) chains with its
output DMAs split across two queues, halving the post-matmul drain.
"""

import numpy as np
import ml_dtypes
from contextlib import ExitStack

import concourse.bass as bass
import concourse.bacc as bacc
import concourse.mybir as mybir
import concourse.tile as tile
from concourse.bass_utils import run_bass_kernel_spmd

B, L, D, C = 2, 512, 768, 256
NCORES = 8
CLOC = C // NCORES          # 32 channels per core
T = B * L                   # 1024 tokens
P = 128
DT = D // P                 # 6 feature tiles
LT = L // P                 # 4 token tiles per batch el
TT = T // P                 # 8 token tiles total
NCH = T // 512              # 2 moving chunks of 512 tokens
NWARM = 26

F32 = mybir.dt.float32
BF = mybir.dt.bfloat16
NBF = ml_dtypes.bfloat16
MODE = "v4-bf16"


def build_program(cloc=CLOC):
    nc = bacc.Bacc("TRN2", target_bir_lowering=False, debug=False)

    # All inputs pre-arranged host-side to [partition, free...] so every
    # DMA is contiguous >=4KB per partition row.
    xTa_h = nc.declare_dram_parameter("xTa", [P, DT * 512], BF, isOutput=False)
    xTb_h = nc.declare_dram_parameter("xTb", [P, DT * 512], BF, isOutput=False)
    sw1T_h = nc.declare_dram_parameter("sw1T", [P, DT * D], BF, isOutput=False)
    # ew1o: ot-major layout [P, ot, kt, 128]
    ew1o_h = nc.declare_dram_parameter("ew1o", [P, DT * D], BF, isOutput=False)
    sb1_h = nc.declare_dram_parameter("sb1", [P, DT], F32, isOutput=False)
    eb1_h = nc.declare_dram_parameter("eb1", [P, DT], F32, isOutput=False)
    what_h = nc.declare_dram_parameter("what", [cloc, P, DT * D], BF, isOutput=False)
    wsT_h = nc.declare_dram_parameter("wsT", [P, DT * cloc], BF, isOutput=False)
    weT_h = nc.declare_dram_parameter("weT", [P, DT * cloc], BF, isOutput=False)
    w0_h = nc.declare_dram_parameter("w0", [cloc, 1], F32, isOutput=False)
    out_h = nc.declare_dram_parameter("out", [B, cloc, L, L], BF, isOutput=True)
    # DRAM bounce for Le rows, so they can be partition-broadcast back in
    leD_h = nc.dram_tensor("leD", [cloc, T], BF)

    Relu = mybir.ActivationFunctionType.Relu
    Ident = mybir.ActivationFunctionType.Identity

    def mm(ps, lhsT, rhs, start, stop):
        nc.tensor.matmul(ps, lhsT, rhs, start=start, stop=stop)

    with tile.TileContext(nc) as tc, ExitStack() as ctx:
        # persistent pools
        p_h = ctx.enter_context(tc.tile_pool(name="hids", bufs=1))
        p_lin = ctx.enter_context(tc.tile_pool(name="lin", bufs=1))

        h1sT = p_h.tile([P, DT, T], BF, tag="hs")
        h1eT = p_h.tile([P, DT, T], BF, tag="he")
        p_w1 = ctx.enter_context(tc.tile_pool(name="w1c", bufs=2))
        what_ap = what_h[:]

        def w1_prefetch(c):
            t = p_w1.tile([P, DT, D], BF, tag="w1t", name="w1t")
            nc.gpsimd.dma_start(out=t[:], in_=what_ap[c])
            return t

        # -------- Phase A0: PE warmup during the DMA lead-in --------
        # N=256 matmuls on a zero tile: ~5.5us of continuous PE busy so
        # the HAM clock-gate (needs ~50 matmuls / >3.4us of activity)
        # has released by the time the critical inputs land (~12.5us).
        # The memset runs on the Vector queue so the gpsimd queue's
        # first instruction is the xA DMA issue.
        wz = p_h.tile([P, 2 * P], BF, tag="wz")
        with tc.tile_pool(name="ps_w", bufs=1, space="PSUM") as ps_w:
            nc.vector.memset(wz[:], 0)
            psw = ps_w.tile([P, 2 * P], F32, tag="psw")
            for i in range(NWARM):
                mm(psw[:], wz[:, :P], wz[:], start=True, stop=True)

        # ---------------- Phase A: layer-1 of both MLPs ----------------
        with (
            tc.tile_pool(name="ph_a", bufs=1) as p_a,
            tc.tile_pool(name="ps_a", bufs=4, space="PSUM") as ps_a,
            tc.tile_pool(name="ps_b", bufs=2, space="PSUM") as ps_b,
            tc.tile_pool(name="bias", bufs=1) as p_bias,
        ):
            # Critical path: xA (gpsimd) + ew1 ot-block tiles (scalar)
            # stream concurrently; everything else queues behind xA on
            # gpsimd so it cannot steal the early HBM bandwidth.
            xA = p_a.tile([P, DT, 512], BF, tag="xa")
            xB = p_a.tile([P, DT, 512], BF, tag="xb")
            # ew1 as three 2-ot-block tiles: the first linear1 chain only
            # gates on xA + block 0 instead of the whole weight matrix.
            ew1 = [p_a.tile([P, 2, DT, P], BF, tag=f"we{g}", name=f"we{g}")
                   for g in range(DT // 2)]
            sw1T = p_a.tile([P, DT, D], BF, tag="sw")
            ew1o_v = ew1o_h[:].rearrange("p (g o t n) -> p g o t n",
                                         g=DT // 2, o=2, t=DT)
            b_sb = {}
            # Everything on ONE queue: per-queue FIFO transfer order is
            # the only reliable HBM-bandwidth priority control.
            nc.gpsimd.dma_start(out=xA[:], in_=xTa_h[:])
            for g in range(DT // 2):
                nc.gpsimd.dma_start(out=ew1[g][:], in_=ew1o_v[:, g])
                if g == 0:
                    for nm, h in (("eb1", eb1_h), ("sb1", sb1_h)):
                        b_sb[nm] = p_bias.tile([P, DT], F32, tag=nm, name=nm)
                        nc.scalar.dma_start(out=b_sb[nm][:], in_=h[:])
            nc.gpsimd.dma_start(out=xB[:], in_=xTb_h[:])
            nc.gpsimd.dma_start(out=sw1T[:], in_=sw1T_h[:])

            def linear1(wt_of, bias_t, outT):
                for xt, off in ((xA, 0), (xB, 512)):
                    for ot in range(DT):
                        ps = ps_a.tile([P, 512], F32, tag="ps_mlp")
                        for kt in range(DT):
                            mm(ps[:],
                               wt_of(ot, kt),
                               xt[:, kt, :],
                               start=(kt == 0), stop=(kt == DT - 1))
                        nc.scalar.activation(
                            outT[:, ot, off:off + 512], ps[:],
                            Relu, bias=bias_t[:, ot:ot + 1])

            # LsP [tok_tile, 128, cloc]  (token on partitions, channel free)
            # LeT [cloc, T] (+ w0)      (channel on partitions, token free)
            lsP = p_lin.tile([P, TT, cloc], F32, tag="lsP")
            leT = p_lin.tile([cloc, T], BF, tag="leT")
            wsT = p_bias.tile([P, DT, cloc], BF, tag="ws")
            weT = p_bias.tile([P, DT, cloc], BF, tag="we")
            w0_sb = p_bias.tile([cloc, 1], F32, tag="w0")

            # End-side first so the Le DRAM bounce round-trip is covered
            # long before the first stage2 consumes it.
            linear1(lambda ot, kt: ew1[ot // 2][:, ot % 2, kt, :],
                    b_sb["eb1"], h1eT)
            nc.scalar.dma_start(out=weT[:], in_=weT_h[:])
            nc.scalar.dma_start(out=w0_sb[:], in_=w0_h[:])
            for chk in range(NCH):
                ps = ps_b.tile([cloc, 512], F32, tag="ps_le")
                for kt in range(DT):
                    mm(ps[:],
                       weT[:, kt, :],
                       h1eT[:, kt, chk * 512:(chk + 1) * 512],
                       start=(kt == 0), stop=(kt == DT - 1))
                nc.scalar.activation(
                    leT[:, chk * 512:(chk + 1) * 512], ps[:], Ident,
                    bias=w0_sb[:, 0:1])
            nc.sync.dma_start(out=leD_h[:], in_=leT[:])

            linear1(lambda ot, kt: sw1T[:, kt, ot * P:(ot + 1) * P],
                    b_sb["sb1"], h1sT)
            nc.scalar.dma_start(out=wsT[:], in_=wsT_h[:])
            w1_pref = [w1_prefetch(0), w1_prefetch(1)]
            for tt_ in range(TT):
                ps = ps_b.tile([P, cloc], F32, tag="ps_ls")
                for kt in range(DT):
                    mm(ps[:],
                       h1sT[:, kt, tt_ * P:(tt_ + 1) * P],
                       wsT[:, kt, :],
                       start=(kt == 0), stop=(kt == DT - 1))
                nc.scalar.activation(lsP[:, tt_, :cloc], ps[:], Ident)

        # ---------------- Phase C: main biaffine loop ----------------
        with (
            tc.tile_pool(name="ttp", bufs=3) as p_tt,
            tc.tile_pool(name="lebp", bufs=4) as p_leb,
            tc.tile_pool(name="outp", bufs=8) as p_out,
            tc.tile_pool(name="ps_s1", bufs=4, space="PSUM") as ps_s1,
            tc.tile_pool(name="ps_s2", bufs=4, space="PSUM") as ps_s2,
        ):
            out_ap = out_h[:]

            def stage1(w1t, b):
                tt_t = p_tt.tile([P, DT, 512], BF, tag="tt", name="tt_t")
                for et in range(DT):
                    ps = ps_s1.tile([P, 512], F32, tag="s1", name="ps1")
                    for dt_ in range(DT):
                        mm(ps[:],
                           w1t[:, dt_, et * P:(et + 1) * P],
                           h1sT[:, dt_, b * 512:(b + 1) * 512],
                           start=(dt_ == 0), stop=(dt_ == DT - 1))
                    nc.vector.tensor_copy(tt_t[:, et, :], ps[:])
                return tt_t

            def stage2(tt_t, c, b, last=False):
                # Le[b,:,c] + w0[c], broadcast across the 128 i-partitions
                leB = p_leb.tile([P, 512], BF, tag="leB", name="leB")
                nc.sync.dma_start(
                    out=leB[:],
                    in_=leD_h[c, b * 512:(b + 1) * 512].partition_broadcast(P))
                # the final iteration runs half-width chains so the
                # post-matmul eviction+DMA tail is half as deep
                nh = 2 if last else 1
                nw = 512 // nh
                for it in range(LT):
                    for h in range(nh):
                        ps2 = ps_s2.tile([P, nw], F32, tag="s2", name="ps2")
                        j0 = b * 512 + h * nw
                        for et in range(DT):
                            mm(ps2[:],
                               tt_t[:, et, it * P:(it + 1) * P],
                               h1eT[:, et, j0:j0 + nw],
                               start=(et == 0), stop=(et == DT - 1))
                        o_t = p_out.tile([P, nw], BF, tag="o", name="o_t")
                        # pass 1 (ACT): psum + Ls -> o_t ; pass 2 (DVE): += Le
                        nc.scalar.activation(
                            o_t[:], ps2[:], Ident,
                            bias=lsP[:, b * LT + it, c:c + 1])
                        nc.vector.tensor_add(
                            o_t[:], o_t[:], leB[:, h * nw:h * nw + nw])
                        q = (nc.sync, nc.scalar)[(it * nh + h) % 2]
                        q.dma_start(
                            out=out_ap[b, c, it * P:(it + 1) * P,
                                       h * nw:h * nw + nw],
                            in_=o_t[:])

            # Software-pipelined: stage1 of iteration k+1 is emitted before
            # stage2 of iteration k so PE never waits on TT evictions.
            pending = None
            for c in range(cloc):
                w1t = w1_pref.pop(0)
                if c + 2 < cloc:
                    w1_pref.append(w1_prefetch(c + 2))
                for b in range(B):
                    tt_t = stage1(w1t, b)
                    if pending is not None:
                        stage2(*pending)
                    pending = (tt_t, c, b)
            stage2(*pending, last=True)
    nc.finalize()
    return nc


def _prep_inputs(inputs, cloc=CLOC, ncores=NCORES):
    """Host-side: absorb layer-2 weights, transpose/cast/shard."""
    f32 = np.float32

    def b16(x):
        return np.ascontiguousarray(np.asarray(x, f32), dtype=NBF)

    def parr(a):
        """[D, N] -> [P, DT*N]: partition-major pre-arranged layout."""
        n = a.shape[1]
        return np.ascontiguousarray(
            a.reshape(DT, P, n).transpose(1, 0, 2).reshape(P, DT * n))

    h = np.asarray(inputs["hidden_states"], f32)
    xT3 = parr(b16(h.reshape(T, D).T)).reshape(P, DT, T)
    xTa = np.ascontiguousarray(xT3[:, :, :512].reshape(P, DT * 512))
    xTb = np.ascontiguousarray(xT3[:, :, 512:].reshape(P, DT * 512))
    sw1T = parr(b16(np.asarray(inputs["sw1"], f32).T))
    # ew1 in ot-major layout: [P, ot, kt, 128]
    ew1_lhsT = b16(np.asarray(inputs["ew1"], f32).T)           # [D_in, D_out]
    ew1o = np.ascontiguousarray(
        ew1_lhsT.reshape(DT, P, DT, P).transpose(1, 2, 0, 3)
        .reshape(P, DT * D))
    sb1 = parr(np.asarray(inputs["sb1"], f32).reshape(D, 1).copy())
    eb1 = parr(np.asarray(inputs["eb1"], f32).reshape(D, 1).copy())
    sw2 = np.asarray(inputs["sw2"], f32)
    ew2 = np.asarray(inputs["ew2"], f32)
    sb2 = np.asarray(inputs["sb2"], f32)
    eb2 = np.asarray(inputs["eb2"], f32)
    W1 = np.asarray(inputs["W1"], f32)
    W2w = np.asarray(inputs["W2_w"], f32)
    Ws, We = W2w[:, :D], W2w[:, D:]

    # Absorb: What[c] = sw2.T @ W1[c] @ ew2, with exact bias folds.
    A = np.matmul(sw2.T[None, :, :], W1)           # [C, D, D]
    What = np.matmul(A, ew2[None, :, :])           # [C, D, D]
    wsT = sw2.T @ Ws.T + (A @ eb2).T               # [D, C]
    weT = ew2.T @ We.T + np.einsum('d,cde->ec', sb2, np.matmul(W1, ew2[None]))
    w0 = (np.asarray(inputs["W2_b"], f32) + np.asarray(inputs["bias"], f32)
          + np.einsum('d,cde,e->c', sb2, W1, eb2)
          + Ws @ sb2 + We @ eb2).reshape(C, 1)

    # What[c]: lhsT layout is [d, e] = What[c] itself; pre-arrange d into
    # [P, DT*D] partition-major per channel.
    What16 = np.ascontiguousarray(
        What.astype(NBF).reshape(C, DT, P, D).transpose(0, 2, 1, 3)
        .reshape(C, P, DT * D))
    wsT16 = parr(b16(wsT))                         # [P, DT*C]
    weT16 = parr(b16(weT))

    wsT16 = wsT16.reshape(P, DT, C)
    weT16 = weT16.reshape(P, DT, C)

    in_maps = []
    for m in range(ncores):
        cs = slice(m * cloc, (m + 1) * cloc)
        in_maps.append({
            "xTa": xTa, "xTb": xTb, "sw1T": sw1T, "ew1o": ew1o,
            "sb1": sb1, "eb1": eb1,
            "what": np.ascontiguousarray(What16[cs]),
            "wsT": np.ascontiguousarray(wsT16[:, :, cs].reshape(P, DT * cloc)),
            "weT": np.ascontiguousarray(weT16[:, :, cs].reshape(P, DT * cloc)),
            "w0": np.ascontiguousarray(w0[cs].astype(f32)),
        })
    return in_maps


def _gather(per_core_outs):
    full = np.concatenate([o.astype(np.float32) for o in per_core_outs],
                          axis=1)                  # [B, C, L, L]
    return np.ascontiguousarray(full.transpose(0, 2, 3, 1))  # [B, L, L, C]


def kernel(**inputs):
    in_maps = _prep_inputs(inputs)
    nc = build_program()
    res = run_bass_kernel_spmd(nc, in_maps, list(range(NCORES)))
    return _gather([np.asarray(r["out"]) for r in res.results])
